# revision 4
# baseline (speedup 1.0000x reference)
"""Trainium2 Bass kernel for nn_CTMDNXCell (scatter_memory).

Strategy (pure data parallel over batch, 8 cores):
  - All state kept transposed in SBUF: [feature partitions, batch free].
  - z_t := s_t * alpha_t  (s_t = 1/(sqrt(beta_t)+1e-6); beta is data
    independent so s_t is host-precomputed per step). Then sync_val == z_t
    and the alpha recurrence becomes z_t = w1_t * z_{t-1} + s_t * pairwise.
  - The act[:, idx] gathers run as indirect DMA row-gathers from a DRAM
    scratch copy of act^T (rows = hidden units, 2KB each).
  - Matmuls in float32r (FP22, full PE rate), accumulate f32 in PSUM.
    x_drive is added into the PSUM group via an identity matmul.
  - The data-dependent early exit (global mean |dh|) is handled by running
    ungated on device while emitting per-step sum(|u|) partial sums; the
    host checks the break condition afterwards. If a break would have
    fired (never for the target inputs; mean|dh| ~ 0.08 >> 0.01), an exact
    numpy fallback recomputes the gated result.
"""

import numpy as np
from contextlib import ExitStack

P = 128
B = 8192
INPUT = 512
HIDDEN = 1024
NSYNCH = 1024
UNFOLDS = 6
DELTA_T = 0.1
ADAPTIVE_THRESHOLD = 0.01
NCORES = 8
B_CORE = B // NCORES  # 1024
PASSES = 2
BP = B_CORE // PASSES  # 512
H_CH = HIDDEN // P  # 8
NS_CH = NSYNCH // P  # 8
IN_CH = INPUT // P  # 4

_f32 = np.float32


def _sigmoid64(v):
    return 1.0 / (1.0 + np.exp(-np.float64(v)))


def _host_scalars(tau_param, r_param, mem_beta_row):
    """Precompute tau, r_sig, beta trajectory, s_t and w1_t (f32)."""
    tau = _f32(np.log1p(np.exp(np.float64(tau_param))) + 0.01)
    r_sig = _sigmoid64(r_param).astype(_f32)  # [NS]
    betas = []  # beta_t for t = 1..UNFOLDS, each [NS]
    svec = []  # s_t = 1/(sqrt(beta_t)+1e-6)
    w1 = []  # w1_t = s_t * r_sig / s_{t-1}, s_0 = 1
    b = mem_beta_row.astype(_f32)
    s_prev = np.ones_like(r_sig)
    for _t in range(UNFOLDS):
        b = r_sig * b + _f32(1.0)
        s = (_f32(1.0) / (np.sqrt(b) + _f32(1e-6))).astype(_f32)
        betas.append(b.copy())
        svec.append(s)
        w1.append((s * r_sig / s_prev).astype(_f32))
        s_prev = s
    return tau, r_sig, betas, svec, w1


def build_nc(bp=BP, passes=PASSES, h_ch=H_CH, ns_ch=NS_CH, in_ch=IN_CH,
             steps=UNFOLDS, neg_inv_tau=-1.0, two_dt=2.0 * DELTA_T):
    """Build the per-core Bass/Tile program. Returns the Bacc object."""
    import concourse.bacc as bacc
    import concourse.tile as tile
    from concourse import bass, mybir
    from concourse.tile_rust import add_dep_helper

    f32 = mybir.dt.float32
    f32r = mybir.dt.float32r
    i32 = mybir.dt.int32
    AF = mybir.ActivationFunctionType
    OP = mybir.AluOpType

    hidden = h_ch * P
    nsynch = ns_ch * P
    inp = in_ch * P
    b_core = bp * passes

    nc = bacc.Bacc("TRN2", target_bir_lowering=False, debug=False)

    # ---- external I/O (per core) ----
    # Everything that feeds the PE is float32r end-to-end (BIR verifier
    # requires producers of fp32r-matmul operands to be fp32r themselves).
    xT_d = nc.dram_tensor("xT", [inp, b_core], f32r, kind="ExternalInput")
    hT_d = nc.dram_tensor("hT", [hidden, b_core], f32r, kind="ExternalInput")
    z0_d = nc.dram_tensor("z0", [P, ns_ch, b_core], f32r, kind="ExternalInput")
    whT_d = nc.dram_tensor("whT", [hidden, hidden], f32r, kind="ExternalInput")
    wmT_d = nc.dram_tensor("wmT", [nsynch, hidden], f32r, kind="ExternalInput")
    wxT_d = nc.dram_tensor("wxT", [inp, hidden], f32r, kind="ExternalInput")
    bh_d = nc.dram_tensor("bh", [P, h_ch], f32, kind="ExternalInput")
    idx_d = nc.dram_tensor("idx", [P, 2 * ns_ch], i32, kind="ExternalInput")
    w1_d = nc.dram_tensor("w1v", [P, steps, ns_ch], f32, kind="ExternalInput")
    sv_d = nc.dram_tensor("sv", [P, steps, ns_ch], f32, kind="ExternalInput")
    ident_d = nc.dram_tensor("ident", [P, P], f32r, kind="ExternalInput")

    ho_d = nc.dram_tensor("ho", [hidden, b_core], f32r, kind="ExternalOutput")
    zo_d = nc.dram_tensor("zo", [P, ns_ch, b_core], f32r, kind="ExternalOutput")
    so_d = nc.dram_tensor("so", [P, passes * steps, h_ch], f32,
                          kind="ExternalOutput")

    # internal DRAM scratch for act^T, one per (pass, step): write once,
    # gather once -- no DRAM WAW hazards at all.
    actT_d = [[nc.dram_tensor(f"actT_{ps}_{t}", [hidden, bp], f32)
               for t in range(steps)] for ps in range(passes)]

    with TileKernel(nc, tile) as tk:
        tc = tk.tc
        ctx = tk.ctx
        wpool = ctx.enter_context(tc.tile_pool(name="weights", bufs=1))
        state = ctx.enter_context(tc.tile_pool(name="state", bufs=1))
        gpool = ctx.enter_context(tc.tile_pool(name="gather", bufs=1))
        apool = ctx.enter_context(tc.tile_pool(name="acts", bufs=3))
        pwpool = ctx.enter_context(tc.tile_pool(name="pw", bufs=3))
        tpool = ctx.enter_context(tc.tile_pool(name="tt", bufs=3))
        upool = ctx.enter_context(tc.tile_pool(name="uu", bufs=3))
        abpool = ctx.enter_context(tc.tile_pool(name="ab", bufs=2))
        psum = ctx.enter_context(tc.tile_pool(name="psum", bufs=8, space="PSUM"))

        # ---- load constants ----
        whT_sb = wpool.tile([P, h_ch, hidden], f32r, tag="whT")
        for k in range(h_ch):
            nc.sync.dma_start(whT_sb[:, k, :], whT_d[k * P:(k + 1) * P, :])
        wmT_sb = wpool.tile([P, ns_ch, hidden], f32r, tag="wmT")
        for c in range(ns_ch):
            nc.sync.dma_start(wmT_sb[:, c, :], wmT_d[c * P:(c + 1) * P, :])
        wxT_sb = wpool.tile([P, in_ch, hidden], f32r, tag="wxT")
        for k in range(in_ch):
            nc.sync.dma_start(wxT_sb[:, k, :], wxT_d[k * P:(k + 1) * P, :])
        bh_sb = wpool.tile([P, h_ch], f32, tag="bh")
        nc.sync.dma_start(bh_sb[:], bh_d[:])
        idx_sb = wpool.tile([P, 2 * ns_ch], i32, tag="idx")
        nc.sync.dma_start(idx_sb[:], idx_d[:])
        w1_sb = wpool.tile([P, steps, ns_ch], f32, tag="w1")
        nc.sync.dma_start(w1_sb[:], w1_d[:])
        sv_sb = wpool.tile([P, steps, ns_ch], f32, tag="sv")
        nc.sync.dma_start(sv_sb[:], sv_d[:])
        ident_sb = wpool.tile([P, P], f32r, tag="ident")
        nc.sync.dma_start(ident_sb[:], ident_d[:])
        sums_sb = wpool.tile([P, passes * steps, h_ch], f32, tag="sums")

        for ps in range(passes):
            bsl = slice(ps * bp, (ps + 1) * bp)
            # per-pass state loads
            xT_sb = state.tile([P, in_ch, bp], f32r, tag="xT")
            for k in range(in_ch):
                nc.sync.dma_start(xT_sb[:, k, :], xT_d[k * P:(k + 1) * P, bsl])
            hT_sb = state.tile([P, h_ch, bp], f32r, tag="h")
            for c in range(h_ch):
                nc.sync.dma_start(hT_sb[:, c, :], hT_d[c * P:(c + 1) * P, bsl])
            z_sb = state.tile([P, ns_ch, bp], f32r, tag="z")
            nc.sync.dma_start(z_sb[:], z0_d[:, :, bsl])
            xd_sb = state.tile([P, h_ch, bp], f32r, tag="xd")

            # x_drive^T = W_x^T.T @ x^T
            for m in range(h_ch):
                pt = psum.tile([P, bp], f32, tag="ps")
                for k in range(in_ch):
                    nc.tensor.matmul(
                        pt[:],
                        wxT_sb[:, k, m * P:(m + 1) * P],
                        xT_sb[:, k, :],
                        start=(k == 0), stop=(k == in_ch - 1))
                nc.vector.tensor_copy(xd_sb[:, m, :], pt[:])

            for t in range(steps):
                # act^T = tanh(h^T) -> DRAM scratch
                st_insts = []
                for c in range(h_ch):
                    a_sb = apool.tile([P, bp], f32, tag="act")
                    nc.scalar.activation(a_sb[:], hT_sb[:, c, :], AF.Tanh)
                    si = nc.sync.dma_start(
                        actT_d[ps][t][c * P:(c + 1) * P, :], a_sb[:])
                    st_insts.append(si)
                # gathers: rows of act^T by idx_left / idx_right.
                # walrus only supports one index per partition per indirect
                # DMA, so issue one call per 128-row chunk.
                gl = gpool.tile([P, ns_ch, bp], f32, tag="gl")
                gr = gpool.tile([P, ns_ch, bp], f32, tag="gr")
                for c in range(ns_ch):
                    gi_l = nc.gpsimd.indirect_dma_start(
                        out=gl[:, c, :], out_offset=None, in_=actT_d[ps][t][:],
                        in_offset=bass.IndirectOffsetOnAxis(
                            ap=idx_sb[:, c:c + 1], axis=0))
                    gi_r = nc.gpsimd.indirect_dma_start(
                        out=gr[:, c, :], out_offset=None, in_=actT_d[ps][t][:],
                        in_offset=bass.IndirectOffsetOnAxis(
                            ap=idx_sb[:, ns_ch + c:ns_ch + c + 1], axis=0))
                    for gi in (gi_l, gi_r):
                        for si in st_insts:
                            add_dep_helper(gi.ins, si.ins,
                                           reason="gather after act store")

                # z update (one chunk at a time; per-partition scalars)
                for c in range(ns_ch):
                    pw = pwpool.tile([P, bp], f32, tag="pw")
                    nc.vector.scalar_tensor_tensor(
                        pw[:], gl[:, c, :], sv_sb[:, t, c:c + 1], gr[:, c, :],
                        op0=OP.mult, op1=OP.mult)
                    nc.vector.scalar_tensor_tensor(
                        z_sb[:, c, :], z_sb[:, c, :], w1_sb[:, t, c:c + 1],
                        pw[:], op0=OP.mult, op1=OP.add)

                # matmuls: psum_m = x_drive + W_h^T.T @ h^T  (+ W_m part)
                pts = []
                for m in range(h_ch):
                    pt = psum.tile([P, bp], f32, tag="ps")
                    pts.append(pt)
                    nc.tensor.matmul(
                        pt[:], ident_sb[:],
                        xd_sb[:, m, :], start=True, stop=False)
                    for k in range(h_ch):
                        nc.tensor.matmul(
                            pt[:],
                            whT_sb[:, k, m * P:(m + 1) * P],
                            hT_sb[:, k, :],
                            start=False, stop=False)
                # W_m part emitted c-outer so PE can start as soon as z[0] is
                # ready
                for c in range(ns_ch):
                    for m in range(h_ch):
                        nc.tensor.matmul(
                            pts[m][:],
                            wmT_sb[:, c, m * P:(m + 1) * P],
                            z_sb[:, c, :],
                            start=False, stop=(c == ns_ch - 1))

                for m in range(h_ch):
                    tt = tpool.tile([P, bp], f32, tag="tt")
                    nc.scalar.activation(tt[:], pts[m][:], AF.Tanh,
                                         bias=bh_sb[:, m:m + 1], scale=1.0)
                    uu = upool.tile([P, bp], f32, tag="uu")
                    nc.vector.scalar_tensor_tensor(
                        uu[:], hT_sb[:, m, :], neg_inv_tau, tt[:],
                        op0=OP.mult, op1=OP.add)
                    ab = abpool.tile([P, bp], f32, tag="ab")
                    nc.scalar.activation(
                        ab[:], uu[:], AF.Abs,
                        accum_out=sums_sb[:, ps * steps + t, m:m + 1])
                    nc.vector.scalar_tensor_tensor(
                        hT_sb[:, m, :], uu[:], two_dt, hT_sb[:, m, :],
                        op0=OP.mult, op1=OP.add)

            # pass outputs
            for c in range(h_ch):
                nc.sync.dma_start(ho_d[c * P:(c + 1) * P, bsl], hT_sb[:, c, :])
            nc.sync.dma_start(zo_d[:, :, bsl], z_sb[:])
        nc.sync.dma_start(so_d[:], sums_sb[:])

    nc.compile()
    return nc


class TileKernel:
    """Small helper: TileContext + ExitStack with ordered teardown."""

    def __init__(self, nc, tile_mod):
        self._tile = tile_mod
        self.nc = nc

    def __enter__(self):
        self.ctx = ExitStack()
        self.ctx.__enter__()
        self.tc = self._tile.TileContext(self.nc)
        # pools must close before the TileContext exits
        self._tc_cm = self.tc
        self._tc_cm.__enter__()
        return self

    def __exit__(self, et, ev, tb):
        try:
            self.ctx.__exit__(et, ev, tb)
        finally:
            return self._tc_cm.__exit__(et, ev, tb)


_NC_CACHE = {}


def _get_nc(neg_inv_tau, two_dt):
    key = (round(float(neg_inv_tau), 12), round(float(two_dt), 12))
    if key not in _NC_CACHE:
        _NC_CACHE[key] = build_nc(neg_inv_tau=neg_inv_tau, two_dt=two_dt)
    return _NC_CACHE[key]


def _prep_inputs(x, h, mem_alpha, W_x, W_h, b_h, idx_left, idx_right,
                 tau, svec, w1):
    """Build the 8 per-core input maps."""
    whT = np.ascontiguousarray(W_h.T.astype(_f32))
    wmT_in = None  # set by caller (needs W_m)
    wxT = np.ascontiguousarray(W_x.T.astype(_f32))
    bh_w = np.ascontiguousarray(b_h.astype(_f32).reshape(H_CH, P).T)
    idx_w = np.concatenate(
        [idx_left.reshape(NS_CH, P).T, idx_right.reshape(NS_CH, P).T],
        axis=1).astype(np.int32)
    idx_w = np.ascontiguousarray(idx_w)
    w1_w = np.ascontiguousarray(
        np.stack([w.reshape(NS_CH, P).T for w in w1], axis=1))  # [P,steps,NS_CH]
    sv_w = np.ascontiguousarray(
        np.stack([s.reshape(NS_CH, P).T for s in svec], axis=1))
    ident = np.eye(P, dtype=_f32)
    shared = dict(whT=whT, wxT=wxT, bh=bh_w, idx=idx_w, w1v=w1_w, sv=sv_w,
                  ident=ident)

    in_maps = []
    for i in range(NCORES):
        rows = slice(i * B_CORE, (i + 1) * B_CORE)
        xT = np.ascontiguousarray(x[rows].T.astype(_f32))
        hT = np.ascontiguousarray(h[rows].T.astype(_f32))
        aT = mem_alpha[rows].T.astype(_f32)  # [NS, B_CORE]
        z0 = np.ascontiguousarray(
            aT.reshape(NS_CH, P, B_CORE).transpose(1, 0, 2))
        in_maps.append(dict(xT=xT, hT=hT, z0=z0, **shared))
    return in_maps


def _numpy_reference(x, h, mem_alpha, mem_beta, W_x, W_h, b_h, tau_param,
                     r_param, W_m, idx_left, idx_right):
    """Exact (gated) f32 reference fallback."""
    tau = _f32(np.log1p(np.exp(np.float64(tau_param))) + 0.01)
    r_sig = _sigmoid64(r_param).astype(_f32)[None, :]
    x32 = x.astype(_f32)
    x_drive = (x32 @ W_x.T.astype(_f32)).astype(_f32)
    h_c = h.astype(_f32).copy()
    a_c = mem_alpha.astype(_f32).copy()
    b_c = mem_beta.astype(_f32).copy()
    done = False
    steps = 0
    for i in range(UNFOLDS):
        if done:
            break
        act = np.tanh(h_c)
        pairwise = act[:, idx_left] * act[:, idx_right]
        new_alpha = r_sig * a_c + pairwise
        new_beta = r_sig * b_c + _f32(1.0)
        sync = new_alpha / (np.sqrt(new_beta) + _f32(1e-6))
        base = x_drive + h_c @ W_h.T.astype(_f32) + b_h.astype(_f32)
        f_t = np.tanh(base + sync @ W_m.T.astype(_f32))
        hu = _f32(DELTA_T) * (-h_c / tau + f_t)
        brk = (i >= 3) and (float(np.abs(hu).mean()) < ADAPTIVE_THRESHOLD)
        h_c = h_c + (_f32(1.0) if brk else _f32(2.0)) * hu
        a_c = new_alpha
        b_c = new_beta
        steps = i
        if brk:
            done = True
    return h_c, a_c, b_c, np.int32(steps)


def kernel(x, h, mem_alpha, mem_beta, W_x, W_h, b_h, tau_param, r_param,
           W_m, idx_left, idx_right):
    from concourse.bass_utils import run_bass_kernel_spmd

    args = dict(x=x, h=h, mem_alpha=mem_alpha, mem_beta=mem_beta, W_x=W_x,
                W_h=W_h, b_h=b_h, tau_param=tau_param, r_param=r_param,
                W_m=W_m, idx_left=idx_left, idx_right=idx_right)
    args = {k: np.asarray(v) for k, v in args.items()}
    x, h = args["x"], args["h"]
    mem_alpha, mem_beta = args["mem_alpha"], args["mem_beta"]
    W_x, W_h, b_h = args["W_x"], args["W_h"], args["b_h"]
    tau_param, r_param, W_m = args["tau_param"], args["r_param"], args["W_m"]
    idx_left = args["idx_left"].astype(np.int64)
    idx_right = args["idx_right"].astype(np.int64)

    # cases the device path does not handle -> exact host fallback
    beta_uniform = bool(np.all(mem_beta == mem_beta[0:1]))
    idx_ok = bool((idx_left >= 0).all() and (idx_left < HIDDEN).all()
                  and (idx_right >= 0).all() and (idx_right < HIDDEN).all())
    if not (beta_uniform and idx_ok and x.shape == (B, INPUT)
            and h.shape == (B, HIDDEN)):
        return _numpy_reference(**args)

    tau, r_sig, betas, svec, w1 = _host_scalars(tau_param, r_param,
                                                mem_beta[0])
    neg_inv_tau = float(-1.0 / tau)
    two_dt = float(2.0 * DELTA_T)
    nc = _get_nc(neg_inv_tau, two_dt)

    in_maps = _prep_inputs(x, h, mem_alpha, W_x, W_h, b_h, idx_left,
                           idx_right, tau, svec, w1)
    wmT = np.ascontiguousarray(W_m.T.astype(_f32))
    for m in in_maps:
        m["wmT"] = wmT

    res = run_bass_kernel_spmd(nc, in_maps, core_ids=list(range(NCORES)))
    outs = res.results

    # ---- assemble outputs ----
    h_f = np.empty((B, HIDDEN), dtype=_f32)
    a_f = np.empty((B, NSYNCH), dtype=_f32)
    sums = np.zeros(UNFOLDS, dtype=np.float64)
    s6 = svec[-1]  # [NS]
    for i in range(NCORES):
        rows = slice(i * B_CORE, (i + 1) * B_CORE)
        o = outs[i]
        h_f[rows] = o["ho"].T
        zT = o["zo"].transpose(1, 0, 2).reshape(NSYNCH, B_CORE)  # [NS, Bc]
        a_f[rows] = zT.T / s6[None, :]
        so = o["so"].astype(np.float64)  # [P, passes*steps, H_CH]
        for t in range(UNFOLDS):
            sums[t] += so[:, t, :].sum() + so[:, UNFOLDS + t, :].sum()

    # early-exit (gating) check on host
    n_el = float(B) * float(HIDDEN)
    brk_step = None
    for t in range(UNFOLDS):
        mean_abs = DELTA_T * sums[t] / n_el
        if t >= 3 and mean_abs < ADAPTIVE_THRESHOLD:
            brk_step = t
            break
    if brk_step is not None:
        # the ungated device result is invalid; recompute exactly
        return _numpy_reference(**args)

    steps = np.int32(UNFOLDS - 1)
    beta6 = betas[-1]  # [NS]
    b_f = np.broadcast_to(beta6[None, :], (B, NSYNCH)).copy()
    return h_f, a_f, b_f, steps
